# revision 4
# baseline (speedup 1.0000x reference)
"""Multi-head spiking (ReLU) attention on 8 Trainium2 NeuronCores.

Sharding: core c -> (batch b = c//4, head-group g = c%4 of 4 heads).
Host pre-transposes x[b] and slices wq/wk/wv column-wise, wo row-wise.
Each core computes its 4 heads' attention matrix (one of the two graded
outputs) and a rank-256 partial of the output projection; the host sums
the four partials per batch and adds bo.

Device dataflow per core (all dims fp32 in HBM):
  qT/kT [dq,S] = (wq slice as stationary) @ xT   (fp32r matmuls)
  v     [S,dq] = xT-chunks as stationary @ wv    (fp32r)
  pass1: logits[q,k] per head -> ReLU eviction -> attn output DMA
  pass2: logits[k,q] -> ReLU + cast fp16 -> attnT tiles
  ctx^T [dq,S]: v^T @ attnT with 2-head column tiling (fp16 matmuls)
  out_part [S,D] = ctx @ wo-slice (fp32r) -> DMA
"""

from dataclasses import dataclass

import numpy as np

import concourse.bass as bass
import concourse.tile as tile
import concourse.mybir as mybir
from concourse import bacc
from concourse.bass_utils import run_bass_kernel_spmd

# Full-problem constants (hardcoded per harness contract).
B, S, D, H = 2, 2048, 1024, 16
DEPTH = D // H  # 64
N_CORES = 8
GROUPS = N_CORES // B       # 4 head-groups
HPC = H // GROUPS           # 4 heads per core
DQ = HPC * DEPTH            # 256 projected dims per core

F32 = mybir.dt.float32
F32R = mybir.dt.float32r
F16 = mybir.dt.float16
RELU = mybir.ActivationFunctionType.Relu
COPY = mybir.ActivationFunctionType.Copy


@dataclass(frozen=True)
class Cfg:
    s: int = S       # sequence length
    d: int = D       # model dim
    dq: int = DQ     # per-core projected dims (HPC*64)

    @property
    def ko(self):
        return self.d // 128   # contraction chunks for projections

    @property
    def mq(self):
        return self.dq // 128  # head-pair chunks (2 heads of depth 64 each)

    @property
    def s5(self):
        return self.s // 512

    @property
    def s1(self):
        return self.s // 128


class EvictBalancer:
    """Distribute PSUM->SBUF evictions between ScalarE and VectorE ~5:4
    (ratio of their element rates)."""

    def __init__(self, nc):
        self.nc = nc
        self.i = 0

    def _use_act(self):
        self.i += 1
        return self.i % 9 < 5

    def relu(self, out, psum):
        if self._use_act():
            self.nc.scalar.activation(out, psum, RELU)
        else:
            self.nc.vector.tensor_scalar_max(out, psum, 0.0)

    def copy(self, out, psum):
        if self._use_act():
            self.nc.scalar.copy(out, psum)
        else:
            self.nc.vector.tensor_copy(out, psum)


def build_nc(cfg: Cfg = Cfg()):
    s, d, dq = cfg.s, cfg.d, cfg.dq
    KO, MQ, S5, S1 = cfg.ko, cfg.mq, cfg.s5, cfg.s1

    nc = bacc.Bacc(None, target_bir_lowering=False)

    xt = nc.dram_tensor("xt", [d, s], F32, kind="ExternalInput")
    wq = nc.dram_tensor("wq", [d, dq], F32, kind="ExternalInput")
    bq = nc.dram_tensor("bq", [dq], F32, kind="ExternalInput")
    wk = nc.dram_tensor("wk", [d, dq], F32, kind="ExternalInput")
    bk = nc.dram_tensor("bk", [dq], F32, kind="ExternalInput")
    wv = nc.dram_tensor("wv", [d, dq], F32, kind="ExternalInput")
    bv = nc.dram_tensor("bv", [dq], F32, kind="ExternalInput")
    wo = nc.dram_tensor("wo", [dq, d], F32, kind="ExternalInput")
    attn_p = nc.dram_tensor("attn_p", [2 * MQ, s, s], F32, kind="ExternalOutput")
    out_p = nc.dram_tensor("out_p", [s, d], F32, kind="ExternalOutput")

    with tile.TileContext(nc) as tc:
        ev = EvictBalancer(nc)
        with tc.tile_pool(name="persist", bufs=1) as pp:
            qT = pp.tile([128, MQ, s], F32R)
            kT = pp.tile([128, MQ, s], F32R)
            vv = pp.tile([128, S1, dq], F16)
            ctxT = pp.tile([128, MQ, s], F32R)
            wo_sb = pp.tile([128, MQ, d], F32R)
            bq_sb = pp.tile([128, MQ], F32)
            bk_sb = pp.tile([128, MQ], F32)
            bv_sb = pp.tile([128, dq], F32)

            nc.gpsimd.dma_start(out=wo_sb, in_=wo.rearrange("(c p) e -> p c e", p=128))
            nc.sync.dma_start(out=bq_sb, in_=bq.rearrange("(c p) -> p c", p=128))
            nc.sync.dma_start(out=bk_sb, in_=bk.rearrange("(c p) -> p c", p=128))
            nc.sync.dma_start(out=bv_sb, in_=bv[None, :].to_broadcast([128, dq]))

            # ---- Phase 1: projections ----
            with (
                tc.tile_pool(name="xw", bufs=1) as xw,
                tc.tile_pool(name="ps_proj", bufs=4, space="PSUM") as psp,
            ):
                xt_sb = xw.tile([128, KO, s], F32R)
                wq_sb = xw.tile([128, KO, dq], F32R)
                wk_sb = xw.tile([128, KO, dq], F32R)
                wv_sb = xw.tile([128, KO, dq], F32R)
                nc.gpsimd.dma_start(
                    out=xt_sb, in_=xt.rearrange("(o p) t -> p o t", p=128)
                )
                nc.gpsimd.dma_start(
                    out=wq_sb, in_=wq.rearrange("(o p) m -> p o m", p=128)
                )
                nc.gpsimd.dma_start(
                    out=wk_sb, in_=wk.rearrange("(o p) m -> p o m", p=128)
                )
                nc.gpsimd.dma_start(
                    out=wv_sb, in_=wv.rearrange("(o p) m -> p o m", p=128)
                )

                # qT / kT: [dq-part, tokens]
                for w_sb, b_sb, dst in ((wq_sb, bq_sb, qT), (wk_sb, bk_sb, kT)):
                    for m in range(MQ):
                        for t in range(S5):
                            ps = psp.tile([128, 512], F32, tag="proj")
                            for o in range(KO):
                                nc.tensor.matmul(
                                    ps,
                                    lhsT=w_sb[:, o, m * 128:(m + 1) * 128],
                                    rhs=xt_sb[:, o, t * 512:(t + 1) * 512],
                                    start=(o == 0),
                                    stop=(o == KO - 1),
                                )
                            nc.vector.tensor_scalar_add(
                                out=dst[:, m, t * 512:(t + 1) * 512],
                                in0=ps,
                                scalar1=b_sb[:, m, None],
                            )

                # v: [token-part, dq]
                for t in range(S1):
                    ps = psp.tile([128, dq], F32, tag="vproj")
                    for o in range(KO):
                        nc.tensor.matmul(
                            ps,
                            lhsT=xt_sb[:, o, t * 128:(t + 1) * 128],
                            rhs=wv_sb[:, o, :],
                            start=(o == 0),
                            stop=(o == KO - 1),
                        )
                    nc.vector.tensor_tensor(
                        vv[:, t, :], ps, bv_sb, mybir.AluOpType.add
                    )

            # ---- Phase 2: attention per head-pair ----
            with (
                tc.tile_pool(name="ps_p1", bufs=3, space="PSUM") as ps1,
                tc.tile_pool(name="ps_p2", bufs=3, space="PSUM") as ps2,
                tc.tile_pool(name="ps_ctx", bufs=2, space="PSUM") as psc,
                tc.tile_pool(name="attn_rows", bufs=3) as rowpool,
                tc.tile_pool(name="attnT", bufs=2) as atpool,
            ):
                for pair in range(MQ):
                    # pass 1: attn rows [q-part, k] -> DRAM
                    for h2 in range(2):
                        h = 2 * pair + h2
                        rows = slice(64 * h2, 64 * (h2 + 1))
                        for qc in range(S1):
                            row = rowpool.tile([128, s], F32, tag="row")
                            for kc in range(S5):
                                ps = ps1.tile([128, 512], F32, tag="p1")
                                nc.tensor.matmul(
                                    ps,
                                    lhsT=qT[rows, pair, qc * 128:(qc + 1) * 128],
                                    rhs=kT[rows, pair, kc * 512:(kc + 1) * 512],
                                )
                                ev.relu(row[:, kc * 512:(kc + 1) * 512], ps)
                            nc.sync.dma_start(
                                out=attn_p[h, qc * 128:(qc + 1) * 128, :], in_=row
                            )

                    # pass 2 + ctx, software-pipelined over q-chunks of 512
                    at_tiles = {}

                    def emit_pass2(j, pair=pair, at_tiles=at_tiles):
                        at = atpool.tile([128, 2, S1, 512], F16, tag="at")
                        at_tiles[j] = at
                        for h2 in range(2):
                            rows = slice(64 * h2, 64 * (h2 + 1))
                            for kc in range(S1):
                                ps = ps2.tile([128, 512], F32, tag="p2")
                                nc.tensor.matmul(
                                    ps,
                                    lhsT=kT[rows, pair, kc * 128:(kc + 1) * 128],
                                    rhs=qT[rows, pair, j * 512:(j + 1) * 512],
                                )
                                ev.relu(at[:, h2, kc, :], ps)

                    def emit_ctx(j, pair=pair, at_tiles=at_tiles):
                        at = at_tiles.pop(j)
                        ps = psc.tile([128, 512], F32, tag="ctx")
                        for kc in range(S1):
                            for h2 in range(2):
                                nc.tensor.matmul(
                                    ps[64 * h2:64 * (h2 + 1), :],
                                    lhsT=vv[:, kc, pair * 128 + 64 * h2:
                                            pair * 128 + 64 * (h2 + 1)],
                                    rhs=at[:, h2, kc, :],
                                    start=(kc == 0),
                                    stop=(kc == S1 - 1),
                                )
                        ev.copy(ctxT[:, pair, j * 512:(j + 1) * 512], ps)

                    emit_pass2(0)
                    for j in range(S5):
                        if j + 1 < S5:
                            emit_pass2(j + 1)
                        emit_ctx(j)

            # ---- Phase 3: output projection ----
            with (
                tc.tile_pool(name="ps_out", bufs=2, space="PSUM") as pso,
                tc.tile_pool(name="out_rows", bufs=2) as outpool,
            ):
                for qc in range(S1):
                    orow = outpool.tile([128, d], F32, tag="orow")
                    for e in range(d // 512):
                        ps = pso.tile([128, 512], F32, tag="po")
                        for m in range(MQ):
                            nc.tensor.matmul(
                                ps,
                                lhsT=ctxT[:, m, qc * 128:(qc + 1) * 128],
                                rhs=wo_sb[:, m, e * 512:(e + 1) * 512],
                                start=(m == 0),
                                stop=(m == MQ - 1),
                            )
                        ev.copy(orow[:, e * 512:(e + 1) * 512], ps)
                    nc.sync.dma_start(
                        out=out_p[qc * 128:(qc + 1) * 128, :], in_=orow
                    )

    nc.finalize()
    return nc


_NC_CACHE = {}


def get_nc(cfg: Cfg = Cfg()):
    if cfg not in _NC_CACHE:
        _NC_CACHE[cfg] = build_nc(cfg)
    return _NC_CACHE[cfg]


LAST_RESULT = None  # BassKernelResults of the most recent kernel() call


def make_in_maps(x, wq, bq, wk, bk, wv, bv, wo):
    scale = 1.0 / np.sqrt(np.float32(DEPTH))
    in_maps = []
    for core in range(N_CORES):
        b, g = divmod(core, GROUPS)
        sl = slice(g * DQ, (g + 1) * DQ)
        in_maps.append({
            "xt": np.ascontiguousarray(x[b].T),
            "wq": np.ascontiguousarray(wq[:, sl]) * scale,
            "bq": np.ascontiguousarray(bq[sl]) * scale,
            "wk": np.ascontiguousarray(wk[:, sl]),
            "bk": np.ascontiguousarray(bk[sl]),
            "wv": np.ascontiguousarray(wv[:, sl]),
            "bv": np.ascontiguousarray(bv[sl]),
            "wo": np.ascontiguousarray(wo[sl, :]),
        })
    return in_maps


def kernel(x, wq, bq, wk, bk, wv, bv, wo, bo, _trace=False):
    global LAST_RESULT
    arrs = [np.asarray(a, np.float32) for a in (x, wq, bq, wk, bk, wv, bv, wo)]
    bo = np.asarray(bo, np.float32)
    in_maps = make_in_maps(*arrs)

    nc = get_nc()
    res = run_bass_kernel_spmd(
        nc, in_maps, core_ids=list(range(N_CORES)), trace=_trace
    )
    LAST_RESULT = res

    attn = np.empty((B, H, S, S), np.float32)
    out = np.broadcast_to(bo, (B, S, D)).copy()
    for core in range(N_CORES):
        b, g = divmod(core, GROUPS)
        r = res.results[core]
        attn[b, g * HPC:(g + 1) * HPC] = r["attn_p"]
        out[b] += r["out_p"]
    return out, attn


# revision 13
# speedup vs baseline: 1.0461x; 1.0461x over previous
"""Multi-head spiking (ReLU) attention on 8 Trainium2 NeuronCores.

Sharding: core c -> (batch b = c//4, head-group g = c%4 of 4 heads).
Host pre-transposes x[b] and slices wq/wk/wv column-wise, wo row-wise.
Each core computes its 4 heads' attention matrix (one of the two graded
outputs) and a rank-256 partial of the output projection; the host sums
the four partials per batch and adds bo.

Device dataflow per core (all tensors fp32 in HBM; fp32r matmuls except
the fp16 context matmuls):
  qT/kT [dq,S] = (wq slice as stationary) @ xT
  v     [S,dq] = xT-chunks as stationary @ wv
  pass1: logits[q,k] per head (row-paired K=64 matmuls) -> ReLU evict
         -> attn output DMA
  pass2: logits[k,q] -> ReLU + cast fp16 -> attnT tiles
  ctx^T [dq,S]: v^T @ attnT with 2-head column tiling (fp16)
  out_part [S,D] = ctx @ wo-slice -> DMA

PSUM is organised as [128,1024] two-bank units so each ACT/DVE eviction
covers 1024 elements (evictions are the phase-2 bottleneck).
"""

from dataclasses import dataclass

import numpy as np

import concourse.bass as bass
import concourse.tile as tile
import concourse.mybir as mybir
from concourse import bacc
from concourse.bass_utils import run_bass_kernel_spmd

# Full-problem constants (hardcoded per harness contract).
B, S, D, H = 2, 2048, 1024, 16
DEPTH = D // H  # 64
N_CORES = 8
GROUPS = N_CORES // B       # 4 head-groups
HPC = H // GROUPS           # 4 heads per core
DQ = HPC * DEPTH            # 256 projected dims per core

F32 = mybir.dt.float32
F32R = mybir.dt.float32r
F16 = mybir.dt.float16
RELU = mybir.ActivationFunctionType.Relu


@dataclass(frozen=True)
class Cfg:
    s: int = S       # sequence length
    d: int = D       # model dim
    dq: int = DQ     # per-core projected dims (HPC*64)

    @property
    def ko(self):
        return self.d // 128   # contraction chunks for projections

    @property
    def mq(self):
        return self.dq // 128  # head-pair chunks (2 heads of depth 64 each)

    @property
    def s5(self):
        return self.s // 512

    @property
    def s1(self):
        return self.s // 128


class EvictBalancer:
    """Distribute PSUM->SBUF evictions between ScalarE and VectorE ~5:4
    (ratio of their element rates)."""

    def __init__(self, nc):
        self.nc = nc
        self.i = 0

    def _use_act(self):
        self.i += 1
        return self.i % 9 < 5

    def relu(self, out, psum):
        if self._use_act():
            self.nc.scalar.activation(out, psum, RELU)
        else:
            self.nc.vector.tensor_scalar_max(out, psum, 0.0)

    def copy(self, out, psum):
        if self._use_act():
            self.nc.scalar.copy(out, psum)
        else:
            self.nc.vector.tensor_copy(out, psum)


def build_nc(cfg: Cfg = Cfg()):
    s, d, dq = cfg.s, cfg.d, cfg.dq
    KO, MQ, S5, S1 = cfg.ko, cfg.mq, cfg.s5, cfg.s1
    S10 = s // 1024

    nc = bacc.Bacc(None, target_bir_lowering=False)

    # fp32r external inputs: bytes are plain fp32; the PE rounds on
    # consumption, and the dtype satisfies the fp32r-rounding verifier
    # without any casting DMA.
    xt = nc.dram_tensor("xt", [d, s], F32R, kind="ExternalInput")
    wq = nc.dram_tensor("wq", [d, dq], F32R, kind="ExternalInput")
    bq = nc.dram_tensor("bq", [dq], F32, kind="ExternalInput")
    wk = nc.dram_tensor("wk", [d, dq], F32R, kind="ExternalInput")
    bk = nc.dram_tensor("bk", [dq], F32, kind="ExternalInput")
    wv = nc.dram_tensor("wv", [d, dq], F32R, kind="ExternalInput")
    bv = nc.dram_tensor("bv", [dq], F32, kind="ExternalInput")
    wo = nc.dram_tensor("wo", [dq, d], F32R, kind="ExternalInput")
    attn_p = nc.dram_tensor("attn_p", [2 * MQ, s, s], F32, kind="ExternalOutput")
    out_p = nc.dram_tensor("out_p", [s, d], F32, kind="ExternalOutput")

    with tile.TileContext(nc) as tc:
        ev = EvictBalancer(nc)
        with tc.tile_pool(name="persist", bufs=1) as pp:
            qT = pp.tile([128, MQ, s], F32R)
            kT = pp.tile([128, MQ, s], F32R)
            vv = pp.tile([128, S1, dq], F16)
            ctxT = pp.tile([128, MQ, s], F32R)
            wo_sb = pp.tile([128, MQ, d], F32R)
            bq_sb = pp.tile([128, MQ], F32)
            bk_sb = pp.tile([128, MQ], F32)
            bv_sb = pp.tile([128, dq], F32)

            nc.sync.dma_start(out=wo_sb, in_=wo.rearrange("(c p) e -> p c e", p=128))
            nc.sync.dma_start(out=bq_sb, in_=bq.rearrange("(c p) -> p c", p=128))
            nc.sync.dma_start(out=bk_sb, in_=bk.rearrange("(c p) -> p c", p=128))
            nc.sync.dma_start(out=bv_sb, in_=bv[None, :].to_broadcast([128, dq]))

            # ---- Phase 1: projections ----
            with (
                tc.tile_pool(name="xw", bufs=1) as xw,
                tc.tile_pool(name="ps_proj", bufs=3, space="PSUM") as psp,
                tc.tile_pool(name="ps_vproj", bufs=2, space="PSUM") as psv,
            ):
                xt_sb = xw.tile([128, KO, s], F32R)
                wq_sb = xw.tile([128, KO, dq], F32R)
                wk_sb = xw.tile([128, KO, dq], F32R)
                wv_sb = xw.tile([128, KO, dq], F32R)
                nc.sync.dma_start(
                    out=xt_sb, in_=xt.rearrange("(o p) t -> p o t", p=128)
                )
                nc.sync.dma_start(
                    out=wq_sb, in_=wq.rearrange("(o p) m -> p o m", p=128)
                )
                nc.sync.dma_start(
                    out=wk_sb, in_=wk.rearrange("(o p) m -> p o m", p=128)
                )
                nc.sync.dma_start(
                    out=wv_sb, in_=wv.rearrange("(o p) m -> p o m", p=128)
                )

                # qT / kT: [dq-part, tokens], 1024-wide PSUM units
                for w_sb, b_sb, dst in ((wq_sb, bq_sb, qT), (wk_sb, bk_sb, kT)):
                    for m in range(MQ):
                        for t in range(S10):
                            ps = psp.tile([128, 1024], F32, tag="proj")
                            for half in range(2):
                                tt = 2 * t + half
                                for o in range(KO):
                                    nc.tensor.matmul(
                                        ps[:, half * 512:(half + 1) * 512],
                                        lhsT=w_sb[:, o, m * 128:(m + 1) * 128],
                                        rhs=xt_sb[:, o, tt * 512:(tt + 1) * 512],
                                        start=(o == 0),
                                        stop=(o == KO - 1),
                                    )
                            nc.vector.tensor_scalar_add(
                                out=dst[:, m, t * 1024:(t + 1) * 1024],
                                in0=ps,
                                scalar1=b_sb[:, m, None],
                            )

                # v: [token-part, dq]
                for t in range(S1):
                    ps = psv.tile([128, dq], F32, tag="vproj")
                    for o in range(KO):
                        nc.tensor.matmul(
                            ps,
                            lhsT=xt_sb[:, o, t * 128:(t + 1) * 128],
                            rhs=wv_sb[:, o, :],
                            start=(o == 0),
                            stop=(o == KO - 1),
                        )
                    nc.vector.tensor_tensor(
                        vv[:, t, :], ps, bv_sb, mybir.AluOpType.add
                    )

            # ---- Phase 2: attention per head-pair ----
            with (
                tc.tile_pool(name="ps_att", bufs=3, space="PSUM") as psa,
                tc.tile_pool(name="ps_ctx", bufs=1, space="PSUM") as psc,
                tc.tile_pool(name="attn_rows", bufs=2) as rowpool,
                tc.tile_pool(name="attnT", bufs=3) as atpool,
            ):
                for pair in range(MQ):
                    # pass 1: attn rows [q-part, k] -> DRAM. Row-paired:
                    # head h2=0 on PE rows 0-63, h2=1 on rows 64-127,
                    # concurrent in the array, separate PSUM units.
                    for qc in range(S1):
                        rows_sb = [
                            rowpool.tile([128, s], F32, tag=f"row{h2}",
                                         name=f"row{h2}_{qc}")
                            for h2 in range(2)
                        ]
                        for kc in range(S10):
                            pss = [
                                psa.tile([128, 1024], F32, tag="att",
                                         name=f"p1_{qc}_{kc}_{h2}")
                                for h2 in range(2)
                            ]
                            for half in range(2):
                                kk = 2 * kc + half
                                for h2 in range(2):
                                    rr = slice(64 * h2, 64 * (h2 + 1))
                                    nc.tensor.matmul(
                                        pss[h2][:, half * 512:(half + 1) * 512],
                                        lhsT=qT[rr, pair, qc * 128:(qc + 1) * 128],
                                        rhs=kT[rr, pair, kk * 512:(kk + 1) * 512],
                                    )
                            for h2 in range(2):
                                ev.relu(
                                    rows_sb[h2][:, kc * 1024:(kc + 1) * 1024],
                                    pss[h2],
                                )
                        for h2 in range(2):
                            nc.sync.dma_start(
                                out=attn_p[2 * pair + h2,
                                           qc * 128:(qc + 1) * 128, :],
                                in_=rows_sb[h2],
                            )

                    # pass 2 + ctx, software-pipelined over q-chunks of 512
                    at_tiles = {}

                    def emit_pass2(j, pair=pair, at_tiles=at_tiles):
                        at = atpool.tile([128, 2, S1, 512], F16, tag="at")
                        at_tiles[j] = at
                        for kc in range(S1 // 2):
                            pss = [
                                psa.tile([128, 1024], F32, tag="att",
                                         name=f"p2_{j}_{kc}_{h2}")
                                for h2 in range(2)
                            ]
                            for half in range(2):
                                kk = 2 * kc + half
                                for h2 in range(2):
                                    rr = slice(64 * h2, 64 * (h2 + 1))
                                    nc.tensor.matmul(
                                        pss[h2][:, half * 512:(half + 1) * 512],
                                        lhsT=kT[rr, pair, kk * 128:(kk + 1) * 128],
                                        rhs=qT[rr, pair, j * 512:(j + 1) * 512],
                                    )
                            # NB: pass2 psum halves are different k-chunks
                            # (2*kc, 2*kc+1) of the SAME head -> contiguous
                            # in at[:, h2, 2kc:2kc+2, :].
                            for h2 in range(2):
                                ev.relu(
                                    at[:, h2, 2 * kc:2 * kc + 2, :].rearrange(
                                        "p a b -> p (a b)"
                                    ),
                                    pss[h2],
                                )

                    def emit_ctx(j2, pair=pair, at_tiles=at_tiles):
                        # one [128,1024] unit = two q-chunks of 512
                        ps = psc.tile([128, 1024], F32, tag="ctx")
                        for jj in range(2):
                            at = at_tiles.pop(2 * j2 + jj)
                            sl = slice(jj * 512, (jj + 1) * 512)
                            for kc in range(S1):
                                for h2 in range(2):
                                    nc.tensor.matmul(
                                        ps[64 * h2:64 * (h2 + 1), sl],
                                        lhsT=vv[:, kc, pair * 128 + 64 * h2:
                                                pair * 128 + 64 * (h2 + 1)],
                                        rhs=at[:, h2, kc, :],
                                        start=(kc == 0),
                                        stop=(kc == S1 - 1),
                                    )
                        ev.copy(ctxT[:, pair, j2 * 1024:(j2 + 1) * 1024], ps)

                    emit_pass2(0)
                    emit_pass2(1)
                    for j2 in range(S5 // 2):
                        emit_ctx(j2)
                        if 2 * j2 + 2 < S5:
                            emit_pass2(2 * j2 + 2)
                        if 2 * j2 + 3 < S5:
                            emit_pass2(2 * j2 + 3)

            # ---- Phase 3: output projection ----
            with (
                tc.tile_pool(name="ps_out", bufs=3, space="PSUM") as pso,
                tc.tile_pool(name="out_rows", bufs=3) as outpool,
            ):
                EW = min(1024, d)
                for qc in range(S1):
                    orow = outpool.tile([128, d], F32, tag="orow")
                    for eu in range(d // EW):
                        ps = pso.tile([128, EW], F32, tag="po")
                        for e2 in range(EW // 512):
                            e = (EW // 512) * eu + e2
                            for m in range(MQ):
                                nc.tensor.matmul(
                                    ps[:, e2 * 512:(e2 + 1) * 512],
                                    lhsT=ctxT[:, m, qc * 128:(qc + 1) * 128],
                                    rhs=wo_sb[:, m, e * 512:(e + 1) * 512],
                                    start=(m == 0),
                                    stop=(m == MQ - 1),
                                )
                        ev.copy(orow[:, eu * EW:(eu + 1) * EW], ps)
                    nc.sync.dma_start(
                        out=out_p[qc * 128:(qc + 1) * 128, :], in_=orow
                    )

    nc.finalize()
    return nc


_NC_CACHE = {}


def get_nc(cfg: Cfg = Cfg()):
    if cfg not in _NC_CACHE:
        _NC_CACHE[cfg] = build_nc(cfg)
    return _NC_CACHE[cfg]


LAST_RESULT = None  # BassKernelResults of the most recent kernel() call


def make_in_maps(x, wq, bq, wk, bk, wv, bv, wo):
    scale = 1.0 / np.sqrt(np.float32(DEPTH))
    in_maps = []
    for core in range(N_CORES):
        b, g = divmod(core, GROUPS)
        sl = slice(g * DQ, (g + 1) * DQ)
        in_maps.append({
            "xt": np.ascontiguousarray(x[b].T),
            "wq": np.ascontiguousarray(wq[:, sl]) * scale,
            "bq": np.ascontiguousarray(bq[sl]) * scale,
            "wk": np.ascontiguousarray(wk[:, sl]),
            "bk": np.ascontiguousarray(bk[sl]),
            "wv": np.ascontiguousarray(wv[:, sl]),
            "bv": np.ascontiguousarray(bv[sl]),
            "wo": np.ascontiguousarray(wo[sl, :]),
        })
    return in_maps


def kernel(x, wq, bq, wk, bk, wv, bv, wo, bo, _trace=False):
    global LAST_RESULT
    arrs = [np.asarray(a, np.float32) for a in (x, wq, bq, wk, bk, wv, bv, wo)]
    bo = np.asarray(bo, np.float32)
    in_maps = make_in_maps(*arrs)

    nc = get_nc()
    res = run_bass_kernel_spmd(
        nc, in_maps, core_ids=list(range(N_CORES)), trace=_trace
    )
    LAST_RESULT = res

    attn = np.empty((B, H, S, S), np.float32)
    out = np.broadcast_to(bo, (B, S, D)).copy()
    for core in range(N_CORES):
        b, g = divmod(core, GROUPS)
        r = res.results[core]
        attn[b, g * HPC:(g + 1) * HPC] = r["attn_p"]
        out[b] += r["out_p"]
    return out, attn
